# revision 1
# baseline (speedup 1.0000x reference)
"""Multi-head attention (B=16, T=1024, D=768, H=12) on 8 TRN2 NeuronCores.

Strategy: pure data parallelism over the batch dim (2 batches per core, no
collectives). Per core, a Tile kernel computes the full attention block:

  qkv = x @ Wqkv.T + b            (q,k produced transposed [o, T]; v normal [T, o])
  scoresT = (k_h qT_h) * scale    ([j, i] layout, 2 heads packed in PE rows)
  expT = exp(scoresT)             (ScalarE, straight from PSUM, fused scale)
  outT_aug = v_aug.T? PV matmul   (v with appended ones col -> rows 0..63 = out,
                                   row 64 = softmax denominators)
  outT = outT_aug[:64] / sums     (DVE recip + K=1 ones-matmul broadcast + mul)
  y = outT.T @ WprojT + b         (normal [t, e] layout, contiguous DMA out)

All matmuls run in bf16 with f32 PSUM accumulation; f32 -> bf16 casts happen
on-chip (DVE). Softmax max-subtraction is skipped: scores are ~N(0,1) here so
exp() cannot overflow f32/bf16.
"""

import numpy as np

import concourse.bass as bass
import concourse.mybir as mybir
import concourse.tile as tile
from concourse import bacc
from concourse.bass_utils import run_bass_kernel_spmd

F32 = mybir.dt.float32
BF16 = mybir.dt.bfloat16

N_CORES = 8
B = 16
T = 1024
NH = 12
HD = 64
DIM = NH * HD
B_LOC = B // N_CORES
TC = 512  # free-dim chunk (one PSUM bank of f32)


def build_nc(b_loc=B_LOC, t=T, nh=NH):
    assert nh % 2 == 0
    dim = nh * HD
    o3 = 3 * dim
    n_dc = dim // 128      # contraction chunks over dim
    n_qk = 2 * dim // 128  # o-tiles covering q and k rows
    n_tt = t // 128        # t tiles
    scale = HD ** -0.5

    nc = bacc.Bacc()

    xT_d = nc.declare_dram_parameter("xT", [b_loc, dim, t], BF16, isOutput=False)
    wq_d = nc.declare_dram_parameter("w_qkvT", [dim, o3], BF16, isOutput=False)
    wp_d = nc.declare_dram_parameter("w_projT", [dim, dim], BF16, isOutput=False)
    bqk_d = nc.declare_dram_parameter("b_qkT", [128, n_qk], F32, isOutput=False)
    bv_d = nc.declare_dram_parameter("b_v", [128, dim], F32, isOutput=False)
    bp_d = nc.declare_dram_parameter("b_proj", [128, dim], F32, isOutput=False)
    out_d = nc.declare_dram_parameter("out", [b_loc, t, dim], F32, isOutput=True)

    with tile.TileContext(nc) as tc:
        with (
            tc.tile_pool(name="wq", bufs=n_dc) as p_wq,
            tc.tile_pool(name="wp", bufs=n_dc) as p_wp,
            tc.tile_pool(name="xbf", bufs=2 * n_dc) as p_x,
            tc.tile_pool(name="qk", bufs=n_qk + 2) as p_qk,
            tc.tile_pool(name="v", bufs=n_tt + 4) as p_v,
            tc.tile_pool(name="outT", bufs=2 * b_loc * (nh // 2)) as p_out,
            tc.tile_pool(name="expT", bufs=12) as p_exp,
            tc.tile_pool(name="bias", bufs=1) as p_b,
            tc.tile_pool(name="y", bufs=3) as p_y,
            tc.tile_pool(name="small", bufs=4) as p_sm,
            tc.tile_pool(name="psmm", bufs=2, space="PSUM") as ps_mm,
            tc.tile_pool(name="pss", bufs=2, space="PSUM") as ps_s,
            tc.tile_pool(name="pso", bufs=4, space="PSUM") as ps_o,
        ):
            # ---- weights (already bf16 from host) / biases ----
            wq_bf = []
            for dc in range(n_dc):
                wb = p_wq.tile([128, o3], BF16, tag="wq")
                nc.sync.dma_start(wb[:], wq_d[dc * 128:(dc + 1) * 128, :])
                wq_bf.append(wb)
            wp_bf = []
            for dc in range(n_dc):
                wb = p_wp.tile([128, dim], BF16, tag="wp")
                nc.sync.dma_start(wb[:], wp_d[dc * 128:(dc + 1) * 128, :])
                wp_bf.append(wb)

            b_qk_sb = p_b.tile([128, n_qk], F32, tag="bqk")
            nc.sync.dma_start(b_qk_sb[:], bqk_d[:, :])
            b_v_sb = p_b.tile([128, dim], F32, tag="bv")
            nc.sync.dma_start(b_v_sb[:], bv_d[:, :])
            b_p_sb = p_b.tile([128, dim], F32, tag="bp")
            nc.sync.dma_start(b_p_sb[:], bp_d[:, :])

            for b in range(b_loc):
                # ---- stage A: qkv projection ----
                x_bf = []
                for dc in range(n_dc):
                    xb = p_x.tile([128, t], BF16, tag="xbf")
                    nc.sync.dma_start(xb[:], xT_d[b, dc * 128:(dc + 1) * 128, :])
                    x_bf.append(xb)

                # q and k, transposed layout [o, t], bias per partition
                qk = []
                for ot in range(n_qk):
                    qt = p_qk.tile([128, t], BF16, tag="qk")
                    for i0 in range(0, t, TC):
                        ic = min(TC, t - i0)
                        ps = ps_mm.tile([128, ic], F32, tag="psmm")
                        for dc in range(n_dc):
                            nc.tensor.matmul(
                                ps[:],
                                lhsT=wq_bf[dc][:, ot * 128:(ot + 1) * 128],
                                rhs=x_bf[dc][:, i0:i0 + ic],
                                start=(dc == 0),
                                stop=(dc == n_dc - 1),
                            )
                        nc.vector.tensor_scalar_add(
                            qt[:, i0:i0 + ic], ps[:], b_qk_sb[:, ot:ot + 1]
                        )
                    qk.append(qt)

                # v, normal layout [t, o'] with a ones column appended per head
                v_tiles = []
                for tt in range(n_tt):
                    vt = p_v.tile([128, nh * 65], BF16, tag="v")
                    v3 = vt[:].rearrange("p (h c) -> p h c", c=65)
                    nc.vector.memset(v3[:, :, 64:65], 1.0)
                    for o0 in range(0, dim, TC):
                        oc = min(TC, dim - o0)
                        h0 = o0 // 64
                        nhc = oc // 64
                        ps = ps_mm.tile([128, oc], F32, tag="psmm")
                        for dc in range(n_dc):
                            nc.tensor.matmul(
                                ps[:],
                                lhsT=x_bf[dc][:, tt * 128:(tt + 1) * 128],
                                rhs=wq_bf[dc][:, 2 * dim + o0:2 * dim + o0 + oc],
                                start=(dc == 0),
                                stop=(dc == n_dc - 1),
                            )
                        nc.vector.tensor_add(
                            v3[:, h0:h0 + nhc, 0:64],
                            ps[:].rearrange("p (h c) -> p h c", c=64),
                            b_v_sb[:, o0:o0 + oc].rearrange("p (h c) -> p h c", c=64),
                        )
                    v_tiles.append(vt)

                # ---- stage B: attention, one head pair at a time ----
                outT = []
                for hp in range(nh // 2):
                    q_tile = qk[hp]
                    k_tile = qk[nh // 2 + hp]
                    o_tile = p_out.tile([128, t], BF16, tag="outT")
                    outT.append(o_tile)
                    for i0 in range(0, t, TC):
                        ic = min(TC, t - i0)
                        po = [
                            ps_o.tile([65, ic], F32, tag="pso", name="po0"),
                            ps_o.tile([65, ic], F32, tag="pso", name="po1"),
                        ]
                        for jt in range(n_tt):
                            pss_pair = []
                            for sub in range(2):
                                pss = ps_s.tile([128, ic], F32, tag="pss")
                                nc.tensor.matmul(
                                    pss[:],
                                    lhsT=k_tile[sub * 64:(sub + 1) * 64,
                                                jt * 128:(jt + 1) * 128],
                                    rhs=q_tile[sub * 64:(sub + 1) * 64, i0:i0 + ic],
                                    start=True,
                                    stop=True,
                                )
                                pss_pair.append(pss)
                            for sub in range(2):
                                h = 2 * hp + sub
                                et = p_exp.tile([128, ic], BF16, tag="expT")
                                nc.scalar.activation(
                                    et[:], pss_pair[sub][:],
                                    mybir.ActivationFunctionType.Exp,
                                    scale=scale,
                                )
                                nc.tensor.matmul(
                                    po[sub][:],
                                    lhsT=v_tiles[jt][:, h * 65:(h + 1) * 65],
                                    rhs=et[:],
                                    start=(jt == 0),
                                    stop=(jt == n_tt - 1),
                                )
                        # normalize: out[d, i] / sums[i]. Emit both recips,
                        # then both broadcasts, then both multiplies — the
                        # broadcast DMA is slow and the DVE runs in order, so
                        # interleaving would serialize recip1 behind a
                        # DMA-waiting multiply.
                        bcs = []
                        for sub in range(2):
                            rec = p_sm.tile([1, ic], F32, tag="recip",
                                            name="rec")
                            nc.vector.reciprocal(rec[:], po[sub][64:65, :])
                            # broadcast 1/sums across the 64 head-dim
                            # partitions via a DMA with a 0-stride free-dim
                            # source AP, keeping the PE fifo free of
                            # normalize work.
                            sb_bc = p_sm.tile([64, ic], F32, tag="bcast",
                                              name="sb_bc")
                            nc.sync.dma_start(
                                sb_bc[:],
                                rec[:].unsqueeze(1).broadcast_to([1, 64, ic]),
                            )
                            bcs.append(sb_bc)
                        for sub in range(2):
                            if sub == 0:
                                nc.vector.tensor_mul(
                                    o_tile[0:64, i0:i0 + ic],
                                    po[sub][0:64, :], bcs[sub][:],
                                )
                            else:
                                tmp = p_sm.tile([64, ic], BF16, tag="ntmp")
                                nc.vector.tensor_mul(tmp[:], po[sub][0:64, :],
                                                     bcs[sub][:])
                                # SWDGE: HWDGE direct2d DMAs carry at most one
                                # sync wait and this copy needs two.
                                nc.gpsimd.dma_start(
                                    o_tile[64:128, i0:i0 + ic], tmp[:]
                                )

                # ---- stage C: output projection ----
                for tt in range(n_tt):
                    for e0 in range(0, dim, TC):
                        ec = min(TC, dim - e0)
                        ps = ps_mm.tile([128, ec], F32, tag="psmm")
                        for dc in range(n_dc):
                            nc.tensor.matmul(
                                ps[:],
                                lhsT=outT[dc][:, tt * 128:(tt + 1) * 128],
                                rhs=wp_bf[dc][:, e0:e0 + ec],
                                start=(dc == 0),
                                stop=(dc == n_dc - 1),
                            )
                        yt = p_y.tile([128, ec], F32, tag="y")
                        nc.vector.tensor_add(yt[:], ps[:], b_p_sb[:, e0:e0 + ec])
                        nc.sync.dma_start(
                            out_d[b, tt * 128:(tt + 1) * 128, e0:e0 + ec], yt[:]
                        )

    nc.compile()
    return nc


def make_in_maps(x, w_qkv, b_qkv, w_proj, b_proj):
    import ml_dtypes

    bf16 = np.dtype(ml_dtypes.bfloat16)
    x = np.asarray(x, dtype=np.float32)
    w_qkvT = np.ascontiguousarray(np.asarray(w_qkv, np.float32).T).astype(bf16)
    w_projT = np.ascontiguousarray(np.asarray(w_proj, np.float32).T).astype(bf16)
    b_qkv = np.asarray(b_qkv, np.float32)
    b_qkT = np.ascontiguousarray(b_qkv[:2 * DIM].reshape(2 * DIM // 128, 128).T)
    b_v = np.ascontiguousarray(np.broadcast_to(b_qkv[2 * DIM:], (128, DIM)))
    b_p = np.ascontiguousarray(np.broadcast_to(np.asarray(b_proj, np.float32), (128, DIM)))
    in_maps = []
    for c in range(N_CORES):
        xs = x[c * B_LOC:(c + 1) * B_LOC]
        xT = np.ascontiguousarray(xs.transpose(0, 2, 1)).astype(bf16)
        in_maps.append({
            "xT": xT,
            "w_qkvT": w_qkvT,
            "w_projT": w_projT,
            "b_qkT": b_qkT,
            "b_v": b_v,
            "b_proj": b_p,
        })
    return in_maps


_NC_CACHE = {}


def _get_nc():
    if "nc" not in _NC_CACHE:
        _NC_CACHE["nc"] = build_nc()
    return _NC_CACHE["nc"]


def run(x, w_qkv, b_qkv, w_proj, b_proj, **rb_kwargs):
    nc = _get_nc()
    in_maps = make_in_maps(x, w_qkv, b_qkv, w_proj, b_proj)
    res = run_bass_kernel_spmd(nc, in_maps, core_ids=list(range(N_CORES)), **rb_kwargs)
    out = np.concatenate([r["out"] for r in res.results], axis=0)
    return out.astype(np.float32), res


def kernel(x, w_qkv, b_qkv, w_proj, b_proj):
    out, _ = run(x, w_qkv, b_qkv, w_proj, b_proj)
    return out

